# revision 2
# baseline (speedup 1.0000x reference)
"""Dot-product attention (B=8, S=2048, DK=DV=512) on 8 TRN2 NeuronCores.

Data-parallel: one batch element per core. Two host-side transforms make the
device kernel fast, both exact:

1. Key compaction. mask_out is a key-padding mask (broadcast over queries),
   known on host: gather the unmasked keys per batch (seed-0 data: 978-1054
   of 2048) and pad to a multiple of 128 (SK=1152). Both matmuls and the
   softmax then run on 9 key chunks instead of 16. Padding keys get K=V=0
   plus an additive -1e4 exp bias, so they contribute exactly 0.

2. bf16 operands. Q^T/K^T/V are converted to bf16 on host; matmuls run in
   bf16 (same 1 row/cycle PE rate as fp32r, but half the DMA/SBUF and no
   on-device dtype conversion passes). exp(scores) is written straight to
   bf16. Measured end-to-end max rel err ~3e-3 vs the f32 reference.

Per core, in transposed-score layout (exp'ed attention chunks are directly
the stationary operand of the second matmul — no attention transposes):

    scores^T[k, q] = K @ Q^T / sqrt(DK)      (PE, bf16, 1 row/cycle)
    attn^T = exp(scores^T + mask_bias[k])    (ACT, fused scale+bias+exp)
    den[q] = sum_k attn^T[k, q]              (DVE chunk-accumulate ->
                                              PE transpose -> DVE reduce;
                                              NO tiny-dim matmuls: M=1/N=1
                                              matmuls cost ~10us each on HW)
    out[q, v] = (attn^T).T @ V / den[q]      (PE accumulate + DVE normalize)

The den transposes are emitted AFTER the first output-subtile matmul block so
the PE never waits on the ACT->DVE accumulation chain. Input DMAs are spread
over the three available queues (SP sync, ACT hwdge, GPSIMD swdge) so the
per-queue descriptor-generation serialization (~0.6us/DMA) stays off the
critical path.
"""

import math
from contextlib import ExitStack

import ml_dtypes
import numpy as np

import concourse.mybir as mybir
import concourse.tile as tile
from concourse import bacc
from concourse.bass_utils import run_bass_kernel_spmd
from concourse.masks import make_identity

B = 8
S = 2048
DK = 512
DV = 512
P = 128
SK = 1152  # padded compacted key length for the seed-0 mask (max count 1054)

F32 = mybir.dt.float32
BF16 = mybir.dt.bfloat16
MASK_BIAS = -10000.0  # exp(bias) == 0 exactly in f32


def build_attention(nc, s=S, dk=DK, dv=DV, sk=SK, q_tile=512, n_reps=1):
    """Emit the per-core attention kernel into `nc` (TileContext inside).

    n_reps > 1 repeats the whole computation (for benchmarking: one NEFF
    timing K serialized executions; output is overwritten each rep).
    """
    scale = 1.0 / math.sqrt(dk)
    nkc = sk // P        # key chunks (partition dim of scores^T)
    ndc = dk // P        # contraction chunks for QK^T
    nqt = s // q_tile    # outer q tiles
    nqs = q_tile // P    # q subtiles per q tile

    # kt plane width: 3 key chunks per DMA when it divides sk, else 1
    kh_w = 3 * P if sk % (3 * P) == 0 else P
    kpg = kh_w // P      # key chunks per kt plane
    nkh = sk // kh_w     # kt planes per d-chunk
    vgs = 3 if nkc % 3 == 0 else 1  # V chunks per DMA group
    nvg = nkc // vgs

    # All inputs are host-prepared: QT/KT pre-transposed [d, seq] bf16,
    # V compacted [sk, dv] bf16, MB additive exp-bias [P, nkc] f32.
    qtd = nc.declare_dram_parameter("QT", [dk, s], BF16, isOutput=False).ap()
    ktd = nc.declare_dram_parameter("KT", [dk, sk], BF16, isOutput=False).ap()
    vd = nc.declare_dram_parameter("V", [sk, dv], BF16, isOutput=False).ap()
    mbd = nc.declare_dram_parameter("MB", [P, nkc], F32, isOutput=False).ap()
    od = nc.declare_dram_parameter("out", [s, dv], F32, isOutput=True).ap()

    with ExitStack() as ctx:
        tc = ctx.enter_context(tile.TileContext(nc))
        constp = ctx.enter_context(tc.tile_pool(name="const", bufs=1))
        # input pools hold 2 reps' tiles so rep r+1 loads overlap rep r compute
        qt0p = ctx.enter_context(tc.tile_pool(name="qt0p", bufs=2 * ndc))
        qtrp = ctx.enter_context(tc.tile_pool(name="qtrp", bufs=2 * ndc))
        ktp = ctx.enter_context(tc.tile_pool(name="ktp", bufs=2 * ndc * nkh))
        vp = ctx.enter_context(tc.tile_pool(name="vp", bufs=2 * nvg))
        attp = ctx.enter_context(tc.tile_pool(name="att", bufs=nkc + 4))
        accp = ctx.enter_context(tc.tile_pool(name="accp", bufs=2))
        denp = ctx.enter_context(tc.tile_pool(name="denp", bufs=4))
        osbp = ctx.enter_context(tc.tile_pool(name="osb", bufs=3))
        stp = ctx.enter_context(tc.tile_pool(name="stp", bufs=3, space="PSUM"))
        opp = ctx.enter_context(tc.tile_pool(name="opp", bufs=3, space="PSUM"))
        dentp = ctx.enter_context(tc.tile_pool(name="dentp", bufs=2, space="PSUM"))

        id_sb = constp.tile([P, P], F32)
        make_identity(nc, id_sb[:])
        mb_sb = constp.tile([P, nkc], F32)
        nc.sync.dma_start(mb_sb[:], mbd[:, :])

        # warm-up exp on a scratch tile: pulls the ~1.3us ACT exp-table load
        # off the critical path (it overlaps the phase-1 DMAs instead of
        # stalling the first score chunk)
        act_warm = constp.tile([P, 1], F32)
        nc.scalar.activation(
            act_warm[:], id_sb[:, 0:1], mybir.ActivationFunctionType.Exp
        )

        # [dk, s] DRAM viewed as [p, dc, q] (partition p within d-chunk)
        qt_view = qtd.rearrange("(dc p) q -> p dc q", p=P)
        kt_view = ktd.rearrange("(dc p) k -> p dc k", p=P)
        v_view = vd.rearrange("(kc p) v -> p kc v", p=P)

        for _rep in range(n_reps):
            qt0 = [qt0p.tile([P, q_tile], BF16, name=f"qt0_{dc}", tag="qt0")
                   for dc in range(ndc)]
            qtr = [qtrp.tile([P, s - q_tile], BF16, name=f"qtr_{dc}", tag="qtr")
                   for dc in range(ndc)]
            kt = [[ktp.tile([P, kh_w], BF16, name=f"kt_{dc}_{h}", tag="kt")
                   for h in range(nkh)] for dc in range(ndc)]
            vg = [vp.tile([P, vgs, dv], BF16, name=f"vg_{g}", tag="vg")
                  for g in range(nvg)]

            # Loads, split by queue so generation (~0.6us each) pipelines:
            #   sync (SP): kt planes (first-needed first), then Q tiles 1-3
            #   scalar (ACT hwdge): Q tile 0 (so mm1 can start ~2.8us in)
            #   gpsimd (swdge): V groups (needed ~10us in; gpsimd is idle)
            for dc in range(ndc):
                nc.sync.dma_start(kt[dc][0][:], kt_view[:, dc, 0:kh_w])
            for dc in range(ndc):
                nc.scalar.dma_start(qt0[dc][:], qt_view[:, dc, 0:q_tile])
            for h in range(1, nkh):
                for dc in range(ndc):
                    nc.sync.dma_start(
                        kt[dc][h][:], kt_view[:, dc, h * kh_w:(h + 1) * kh_w]
                    )
            for g in range(nvg):
                nc.gpsimd.dma_start(vg[g][:], v_view[:, g * vgs:(g + 1) * vgs, :])
            for dc in range(ndc):
                nc.sync.dma_start(qtr[dc][:], qt_view[:, dc, q_tile:s])

            def v_sl(kc):
                return vg[kc // vgs][:, kc % vgs, :]

            for qt_i in range(nqt):
                if qt_i == 0:
                    qsrc = [qt0[dc][:] for dc in range(ndc)]
                else:
                    qsrc = [
                        qtr[dc][:, (qt_i - 1) * q_tile:qt_i * q_tile]
                        for dc in range(ndc)
                    ]

                # mm1 + exp + den accumulation over key chunks
                at_tiles = []
                acc = accp.tile([P, q_tile], F32, tag="acc")
                for kc in range(nkc):
                    st = stp.tile([P, q_tile], F32)
                    for dc in range(ndc):
                        nc.tensor.matmul(
                            st[:],
                            kt[dc][kc // kpg][:, (kc % kpg) * P:(kc % kpg + 1) * P],
                            qsrc[dc],
                            start=(dc == 0),
                            stop=(dc == ndc - 1),
                        )
                    at = attp.tile([P, q_tile], BF16, tag="at")
                    nc.scalar.activation(
                        at[:],
                        st[:],
                        mybir.ActivationFunctionType.Exp,
                        bias=mb_sb[:, kc:kc + 1],
                        scale=scale,
                    )
                    at_tiles.append(at)
                    if kc == 0:
                        nc.vector.tensor_copy(acc[:], at[:])
                    else:
                        nc.vector.tensor_add(acc[:], acc[:], at[:])

                def mm2(qs):
                    op = opp.tile([P, dv], F32)
                    for kc in range(nkc):
                        nc.tensor.matmul(
                            op[:],
                            at_tiles[kc][:, qs * P:(qs + 1) * P],
                            v_sl(kc),
                            start=(kc == 0),
                            stop=(kc == nkc - 1),
                        )
                    return op

                def finish(qs, op):
                    ob = osbp.tile([P, dv], F32, tag="ob")
                    nc.vector.tensor_scalar_mul(
                        ob[:], op[:], recip_sb[:, qs:qs + 1]
                    )
                    nc.scalar.dma_start(
                        od[qt_i * q_tile + qs * P:qt_i * q_tile + (qs + 1) * P, :],
                        ob[:],
                    )

                # first output subtile's matmuls go ahead of the den
                # transposes so the PE isn't waiting on the exp/acc chain
                op0 = mm2(0)

                # den[q] = partition-sum of acc: PE-transpose the 4 q-slices
                # into one PSUM bank, one 3D free-dim reduce, one reciprocal.
                dent_ps = dentp.tile([P, nqs * P], F32)
                for qs in range(nqs):
                    nc.tensor.transpose(
                        dent_ps[:, qs * P:(qs + 1) * P],
                        acc[:, qs * P:(qs + 1) * P],
                        id_sb[:],
                    )
                den_sb = denp.tile([P, nqs], F32, tag="den")
                nc.vector.reduce_sum(
                    den_sb[:],
                    dent_ps[:].rearrange("p (qs k) -> p qs k", qs=nqs),
                    axis=mybir.AxisListType.X,
                )
                recip_sb = denp.tile([P, nqs], F32, tag="recip")
                nc.vector.reciprocal(recip_sb[:], den_sb[:])

                finish(0, op0)
                for qs in range(1, nqs):
                    finish(qs, mm2(qs))


def make_device_inputs(Q, K, V, mask_out, sk=None):
    """Host prep: compact keys per batch, transpose Q/K, convert to bf16.

    Returns (in_maps, sk): one {QT, KT, V, MB} dict per core and the padded
    key length actually used (ceil(max unmasked count / 128) * 128).
    """
    Q = np.asarray(Q, dtype=np.float32)
    K = np.asarray(K, dtype=np.float32)
    V = np.asarray(V, dtype=np.float32)
    m = np.asarray(mask_out).reshape(B, S).astype(bool)
    keep = ~m
    counts = keep.sum(axis=1)
    if sk is None:
        sk = max(P, int(-(-int(counts.max()) // P)) * P)
    nkc = sk // P
    bf16 = ml_dtypes.bfloat16

    in_maps = []
    for b in range(B):
        idx = np.nonzero(keep[b])[0]
        nk = len(idx)
        assert nk <= sk, (nk, sk)
        kt = np.zeros((DK, sk), dtype=bf16)
        kt[:, :nk] = K[b][idx].T.astype(bf16)
        vc = np.zeros((sk, DV), dtype=bf16)
        vc[:nk] = V[b][idx].astype(bf16)
        mb = np.zeros(sk, dtype=np.float32)
        mb[nk:] = MASK_BIAS
        in_maps.append({
            "QT": np.ascontiguousarray(Q[b].T).astype(bf16),
            "KT": np.ascontiguousarray(kt),
            "V": np.ascontiguousarray(vc),
            # [P, nkc] chunk layout: MB[p, kc] = bias for key kc*128+p
            "MB": np.ascontiguousarray(mb.reshape(nkc, P).T),
        })
    return in_maps, sk


_CACHE = {}


def _get_compiled(sk):
    if sk not in _CACHE:
        nc = bacc.Bacc(
            "TRN2", target_bir_lowering=False, debug=False, num_devices=B
        )
        build_attention(nc, sk=sk)
        nc.compile()
        _CACHE[sk] = nc
    return _CACHE[sk]


def run(Q, K, V, mask_out, **spmd_kwargs):
    """Returns (full_output, BassKernelResults)."""
    in_maps, sk = make_device_inputs(Q, K, V, mask_out)
    nc = _get_compiled(sk)
    res = run_bass_kernel_spmd(nc, in_maps, list(range(B)), **spmd_kwargs)
    out = np.stack([res.results[b]["out"] for b in range(B)]).astype(np.float32)
    return out, res


def kernel(Q, K, V, mask_out):
    return run(Q, K, V, mask_out)[0]
